# revision 1
# baseline (speedup 1.0000x reference)
"""GCN embedder kernel for TRN2, 8-core SPMD.

Design
------
* Nodes sharded contiguously across C=8 cores (NC nodes each). Edges
  (incl. self-loops) are owned by the dst core.
* Node features h are kept feature-major in SBUF: hT [H=128 part, NC].
* Per layer l:
    - (l>=2) hwT = matmul(lhsT=W_l, rhs=hT) per 512-col chunk; transpose
      each 128-col piece to node-major, DMA to hw_shard dram [NC, H];
      AllGather -> hw_full [N, H] (the gather table).
    - (l==1) the gather table is T1 = emb @ W1 (only 128 rows, computed
      once at the start; gather index is x[src]).
    - Edge pass: edges sorted by (src chunk, dst). dma_gather fetches
      128-edge message tiles M [128e, H] from the layer's table
      (int16 indices => table split in CH=4 chunks of N/4 rows).
      One DVE tensor_scalar builds the norm-scaled one-hot indicator
      B[e, d] = (iota[d] == dstrel[e]) * norm[e]   [128e, 128d]
      and matmul(lhsT=M, rhs=B) accumulates hT_win [H, 128d] in PSUM over
      the tiles of each dst window (128 dst nodes, PSUM quad [128,512]
      packs 4 windows/bank).  Window flush:
        l1:  ACT relu(psum + b1) -> hT buffer (single pass, fused)
        l2/3: DVE add psum into an accumulator; after the last chunk a
              final ACT pass applies bias (+relu for l2) in place.
* Pooling: transpose h3 tiles to node-major; indicator matmul against
  batchrel one-hot accumulates pooledT [H, <=256 graphs] in PSUM (each
  core's nodes span <=256 graph ids); transpose back, scatter rows by
  graph id (indirect DMA) into a zeroed [G+256, H] buffer; AllReduce;
  multiply by host-precomputed 1/cnt; core 0's output is the answer.

All structure (tile counts per window, gather call sizes) is maxed
across cores so the single SPMD program fits every core; pad edges have
norm=0 (indicator kills their contribution) and index 0 (valid row).
"""

import math
from contextlib import ExitStack
from dataclasses import dataclass, field

import numpy as np

import concourse.mybir as mybir
import concourse.tile as tile
from concourse import bacc, bass
from concourse.bass import AP, IndirectOffsetOnAxis, ds
from concourse.masks import make_identity

F32 = mybir.dt.float32
I16 = mybir.dt.int16
I32 = mybir.dt.int32
AF = mybir.ActivationFunctionType
OP = mybir.AluOpType

P = 128  # partitions / hidden size / vocab


@dataclass
class Cfg:
    N: int = 100000
    E: int = 1600000
    H: int = 128
    V: int = 128
    L: int = 3
    G: int = 1024
    C: int = 8          # cores
    CH: int = 4         # gather-table chunks (int16 index limit)
    TPC: int = 32       # max tiles per dma_gather call

    @property
    def NC(self):  # nodes per core
        assert self.N % self.C == 0
        return self.N // self.C

    @property
    def CHN(self):  # rows per gather chunk
        assert self.N % self.CH == 0
        return self.N // self.CH

    @property
    def W(self):  # dst windows per core
        return math.ceil(self.NC / P)

    @property
    def NCP(self):  # padded nodes per core (to window multiple)
        return self.W * P

    @property
    def GSPAN(self):  # pooling graph-window width per core
        return 256


@dataclass
class Structure:
    """Uniform-across-cores program structure."""
    # layer-1 stream: per window w, number of 128-edge tiles
    t1_w: list = field(default_factory=list)
    # layer-2/3 stream: per (chunk, window), number of tiles
    t23_kw: list = field(default_factory=list)  # [CH][W]
    # gather calls: list of (tile_start, n_tiles) for l1; per chunk for l23
    calls1: list = field(default_factory=list)
    calls23: list = field(default_factory=list)  # [(chunk, tile_start, n_tiles)]

    @property
    def T1(self):
        return sum(self.t1_w)

    @property
    def T23(self):
        return sum(sum(r) for r in self.t23_kw)


def _pad_groups(order_keys, group_ids, n_groups):
    """Given sorted group id per edge (in stream order), return
    counts per group."""
    cnt = np.bincount(group_ids, minlength=n_groups)
    return cnt


def _chop_calls(total_tiles, tpc):
    calls = []
    t = 0
    while t < total_tiles:
        n = min(tpc, total_tiles - t)
        calls.append((t, n))
        t += n
    return calls


def preprocess(x, edge_index, batch, emb_table, Ws, bs, cfg: Cfg):
    """Host-side (index-only) preprocessing.

    Returns (structure, per-core input maps (numpy arrays), aux info).
    """
    N, E, C, CH = cfg.N, cfg.E, cfg.C, cfg.CH
    NC, CHN, W = cfg.NC, cfg.CHN, cfg.W

    x = np.asarray(x).astype(np.int64)
    edge_index = np.asarray(edge_index).astype(np.int64)
    batch = np.asarray(batch).astype(np.int64)
    emb_table = np.asarray(emb_table, dtype=np.float32)
    Ws = np.asarray(Ws, dtype=np.float32)
    bs = np.asarray(bs, dtype=np.float32)

    loop = np.arange(N, dtype=np.int64)
    src = np.concatenate([edge_index[0], loop])
    dst = np.concatenate([edge_index[1], loop])
    deg = np.bincount(dst, minlength=N).astype(np.float32)
    dinv = 1.0 / np.sqrt(deg)  # deg >= 1 thanks to self loops
    norm = (dinv[src] * dinv[dst]).astype(np.float32)
    xsrc = x[src]

    owner = dst // NC

    # ---- per-core streams ----
    dinv2 = (dinv * dinv).astype(np.float32)
    per_core = []
    for c in range(C):
        m = owner == c
        s_c, d_c, n_c, xs_c = src[m], dst[m] - c * NC, norm[m], xsrc[m]
        # layer-1 stream: sort by dst (includes self loops)
        o1 = np.argsort(d_c, kind="stable")
        # layer-2/3 stream: non-loop edges sorted by (chunk, dst), then a
        # synthetic "self" chunk (k=CH) of exactly one tile per window that
        # gathers from the local hw_shard.
        m23 = owner[:E] == c
        s23 = src[:E][m23]
        d23 = dst[:E][m23] - c * NC
        n23 = norm[:E][m23]
        ck23 = s23 // CHN
        vloc = np.arange(NC, dtype=np.int64)
        s23 = np.concatenate([s23, vloc])          # srel handled below
        d23 = np.concatenate([d23, vloc])
        n23 = np.concatenate([n23, dinv2[c * NC + vloc]])
        ck23 = np.concatenate([ck23, np.full(NC, CH, np.int64)])
        o23 = np.lexsort((d23, ck23))
        srel23 = np.where(ck23 == CH, s23, s23 - ck23 * CHN)
        per_core.append(dict(
            s=s_c, d=d_c, n=n_c, xs=xs_c, o1=o1,
            s23=srel23, d23=d23, n23=n23, ck23=ck23, o23=o23))

    # ---- uniform tile counts ----
    CHX = CH + 1
    t1_w = np.zeros(W, dtype=np.int64)
    t23_kw = np.zeros((CHX, W), dtype=np.int64)
    for c in range(C):
        pc = per_core[c]
        d1 = pc["d"][pc["o1"]]
        w1 = d1 // P
        cnt1 = np.bincount(w1, minlength=W)
        t1_w = np.maximum(t1_w, -(-cnt1 // P))
        dk = pc["d23"][pc["o23"]]
        kk = pc["ck23"][pc["o23"]]
        gid = kk * W + dk // P
        cntk = np.bincount(gid, minlength=CHX * W).reshape(CHX, W)
        t23_kw = np.maximum(t23_kw, -(-cntk // P))
    assert (t1_w >= 1).all()
    assert (t23_kw[CH] == 1).all()

    st = Structure(t1_w=list(t1_w), t23_kw=[list(r) for r in t23_kw])
    T1, T23 = st.T1, st.T23
    st.calls1 = _chop_calls(T1, cfg.TPC)
    st.calls23 = []
    toff = 0
    for k in range(CHX):
        tk = sum(st.t23_kw[k])
        for (t0, nt) in _chop_calls(tk, cfg.TPC):
            st.calls23.append((k, toff + t0, nt))
        toff += tk

    # ---- build padded per-core arrays ----
    def build_stream(d_sorted, n_sorted, idx_sorted, group_of_edge, counts_T,
                     idx_dtype):
        """Pack a sorted edge stream into per-group padded tiles.

        group_of_edge: group id per edge (sorted, contiguous)
        counts_T: tiles per group (uniform)
        returns meta [P, Ttot, 2] f32, idx_stream [Ttot*128] idx_dtype
        """
        n_groups = len(counts_T)
        Ttot = int(sum(counts_T))
        meta = np.zeros((P, Ttot, 2), dtype=np.float32)
        idxs = np.zeros(Ttot * P, dtype=idx_dtype)
        # boundaries of groups in the edge stream
        cnt = np.bincount(group_of_edge, minlength=n_groups)
        starts = np.concatenate([[0], np.cumsum(cnt)[:-1]])
        t0 = 0
        for g in range(n_groups):
            cg, sg, Tg = int(cnt[g]), int(starts[g]), int(counts_T[g])
            assert cg <= Tg * P, (g, cg, Tg)
            sl = slice(sg, sg + cg)
            eoff = t0 * P
            # slot i (within group) -> tile t0 + i//128, partition i%128
            ii = np.arange(cg)
            tt = t0 + ii // P
            pp = ii % P
            meta[pp, tt, 0] = (d_sorted[sl] % P).astype(np.float32)
            meta[pp, tt, 1] = n_sorted[sl]
            idxs[eoff + ii] = idx_sorted[sl].astype(idx_dtype)
            t0 += Tg
        return meta, idxs

    def wrap_idxs(idx_stream, calls, nidx_cols):
        """Wrap per-call indices into the dma_gather [128, cols] layout."""
        out = np.zeros((P, nidx_cols), dtype=np.int16)
        col = 0
        for (t0, nt) in [(a, b) for (a, b) in calls]:
            arr = idx_stream[t0 * P:(t0 + nt) * P]
            wr = arr.reshape(-1, 16).T  # [16, nt*8]
            out[:, col:col + nt * 8] = np.tile(wr, (8, 1))
            col += nt * 8
        assert col == nidx_cols
        return out

    in_maps = []
    gmins = []
    for c in range(C):
        pc = per_core[c]
        o1, o23 = pc["o1"], pc["o23"]
        d1, n1, xs1 = pc["d"][o1], pc["n"][o1], pc["xs"][o1]
        w1 = d1 // P
        meta1, idx1 = build_stream(d1, n1, xs1, w1, t1_w, np.int16)

        dk, nk, srel, kk = (pc["d23"][o23], pc["n23"][o23],
                            pc["s23"][o23], pc["ck23"][o23])
        gidk = kk * W + dk // P
        assert (srel >= 0).all() and (srel < CHN).all()
        meta23, idx23 = build_stream(dk, nk, srel, gidk,
                                     t23_kw.reshape(-1), np.int16)

        gidx1 = wrap_idxs(idx1, st.calls1, T1 * 8)
        gidx23 = wrap_idxs(idx23, [(t0, nt) for (_k, t0, nt) in st.calls23],
                           T23 * 8)

        # pooling metadata
        nodes = np.arange(cfg.NCP) + c * NC
        valid = nodes < (c + 1) * NC
        bvals = np.where(valid, batch[np.minimum(nodes, N - 1)], -1)
        gmin = int(batch[c * NC])
        gmax = int(batch[min((c + 1) * NC, N) - 1])
        assert gmax - gmin < cfg.GSPAN, (c, gmin, gmax)
        brel = np.where(valid, bvals - gmin, -1).astype(np.float32)
        # [P, W] node tile layout: node = tile*128 + p
        pool_meta = brel.reshape(cfg.W, P).T.copy()  # [128, W]
        # scatter row ids: rows gmin..gmin+255 ; out of range -> trash rows
        gid_rows = gmin + np.arange(cfg.GSPAN)
        gid_rows = np.where(gid_rows < cfg.G, gid_rows,
                            cfg.G + np.arange(cfg.GSPAN) % 256).astype(np.int32)
        # [128, 2] int32 (two scatter calls of 128 rows)
        gid_cols = gid_rows.reshape(2, P).T.copy()

        cnts = np.bincount(batch, minlength=cfg.G).astype(np.float32)
        recip = 1.0 / np.maximum(cnts, 1.0)
        recip_pm = recip.reshape(cfg.G // P, P).T.copy()  # [128, G/128]

        in_maps.append({
            "meta1": meta1, "gidx1": gidx1,
            "meta23": meta23, "gidx23": gidx23,
            "pool_meta": pool_meta, "gid_cols": gid_cols,
            "recip_pm": recip_pm,
            "emb": emb_table, "Ws": Ws, "bs": bs,
        })
        gmins.append(gmin)

    return st, in_maps, gmins


# --------------------------------------------------------------------------
# device program
# --------------------------------------------------------------------------

def build_nc(cfg: Cfg, st: Structure, stage: int = 99):
    N, H, C, CH, W = cfg.N, cfg.H, cfg.C, cfg.CH, cfg.W
    NC, CHN, NCP = cfg.NC, cfg.CHN, cfg.NCP
    T1, T23 = st.T1, st.T23
    GS = cfg.GSPAN
    GW = cfg.G // P  # graph windows in output

    nc = bacc.Bacc(None, num_devices=C, num_swdge_queues=2)
    cores = list(range(C))

    # ---- external I/O ----
    meta1 = nc.declare_dram_parameter("meta1", [P, T1, 2], F32, isOutput=False)
    gidx1 = nc.declare_dram_parameter("gidx1", [P, T1 * 8], I16, isOutput=False)
    meta23 = nc.declare_dram_parameter("meta23", [P, T23, 2], F32, isOutput=False)
    gidx23 = nc.declare_dram_parameter("gidx23", [P, T23 * 8], I16, isOutput=False)
    pool_meta = nc.declare_dram_parameter("pool_meta", [P, W], F32, isOutput=False)
    gid_cols = nc.declare_dram_parameter("gid_cols", [P, 2], I32, isOutput=False)
    recip_pm = nc.declare_dram_parameter("recip_pm", [P, GW], F32, isOutput=False)
    emb_d = nc.declare_dram_parameter("emb", [P, H], F32, isOutput=False)
    Ws_d = nc.declare_dram_parameter("Ws", [cfg.L, H, H], F32, isOutput=False)
    bs_d = nc.declare_dram_parameter("bs", [cfg.L, H], F32, isOutput=False)
    out_d = nc.declare_dram_parameter("out", [cfg.G, H], F32, isOutput=True)

    # ---- internal DRAM ----
    t1_dram = nc.dram_tensor("t1_tab", [cfg.V, H], F32)
    hw_shard = nc.dram_tensor("hw_shard", [NC, H], F32)
    hw_full = nc.dram_tensor("hw_full", [N, H], F32, addr_space="Shared")
    pooled_nm = nc.dram_tensor("pooled_nm", [cfg.G + GS, H], F32)
    pooled_sum = nc.dram_tensor("pooled_sum", [cfg.G + GS, H], F32,
                                addr_space="Shared")

    from concourse.tile import add_dep_helper
    pd = {"i": 0, "last": None}

    def chain_pool_dma(inst):
        if pd["last"] is not None:
            add_dep_helper(inst.ins, pd["last"].ins, sync=False,
                           reason="pool-dma queue/lane parity order")
        pd["last"] = inst
        pd["i"] += 1

    with tile.TileContext(nc) as tc, ExitStack() as ctx:
        const = ctx.enter_context(tc.tile_pool(name="const", bufs=1))
        hpool = ctx.enter_context(tc.tile_pool(name="hbuf", bufs=1))

        ident = const.tile([P, P], F32)
        make_identity(nc, ident[:])
        iota_i = const.tile([P, GS], I32)
        nc.gpsimd.iota(iota_i[:], pattern=[[1, GS]], base=0,
                       channel_multiplier=0)
        iota_f = const.tile([P, GS], F32)
        nc.vector.tensor_copy(out=iota_f[:], in_=iota_i[:])

        w_sb = const.tile([P, H], F32, tag="w_sb")
        emb_sb = const.tile([P, H], F32, tag="w_sb2")
        b_cols = const.tile([P, cfg.L], F32)
        for l in range(cfg.L):
            nc.sync.dma_start(out=b_cols[:, l:l + 1], in_=bs_d[l, :, None])

        hT_a = hpool.tile([P, NCP], F32)   # layer outputs (ping)
        hT_b = hpool.tile([P, NCP], F32)   # (pong)

        # ---------------- T1 = emb @ W1 ----------------
        with tc.tile_pool(name="pro", bufs=2) as pro, \
             tc.tile_pool(name="pro_ps", bufs=2, space="PSUM") as pro_ps:
            nc.sync.dma_start(out=emb_sb[:], in_=emb_d[:, :])
            nc.sync.dma_start(out=w_sb[:], in_=Ws_d[0])
            embT_ps = pro_ps.tile([P, P], F32)
            nc.tensor.transpose(out=embT_ps[:], in_=emb_sb[:], identity=ident[:])
            embT = pro.tile([P, P], F32)
            nc.vector.tensor_copy(out=embT[:], in_=embT_ps[:])
            t1t_ps = pro_ps.tile([P, P], F32)
            nc.tensor.matmul(out=t1t_ps[:], lhsT=w_sb[:], rhs=embT[:],
                             start=True, stop=True)
            t1t = pro.tile([P, P], F32)
            nc.vector.tensor_copy(out=t1t[:], in_=t1t_ps[:])
            t1nm_ps = pro_ps.tile([P, P], F32)
            nc.tensor.transpose(out=t1nm_ps[:], in_=t1t[:], identity=ident[:])
            t1nm = pro.tile([P, P], F32)
            nc.vector.tensor_copy(out=t1nm[:], in_=t1nm_ps[:])
            nc.sync.dma_start(out=t1_dram[:, :], in_=t1nm[:])

        # ---------------- layer loop ----------------
        def edge_pass(layer, h_out, h_acc):
            """layer: 0,1,2. h_out: hT tile written.
            h_acc: for layers>=1 the accumulation buffer (== h_out)."""
            l1 = layer == 0
            meta_d, gidx_d = (meta1, gidx1) if l1 else (meta23, gidx23)
            calls = [(0, t0, nt) for (t0, nt) in st.calls1] if l1 \
                else st.calls23
            tw = ([(0, w, st.t1_w[w]) for w in range(W)] if l1 else
                  [(k, w, st.t23_kw[k][w]) for k in range(CH + 1)
                   for w in range(W)])
            table = t1_dram if l1 else hw_full

            with tc.tile_pool(name=f"ep{layer}", bufs=3) as ep, \
                 tc.tile_pool(name=f"gb{layer}", bufs=2) as gb, \
                 tc.tile_pool(name=f"bq{layer}", bufs=4) as bq, \
                 tc.tile_pool(name=f"eps{layer}", bufs=3, space="PSUM") as eps:

                if not l1:
                    nc.vector.memset(h_acc[:], 0.0)

                # gather calls -> gathered tile buffers, keyed by call idx
                gbuf = {}
                for ci, (k, t0, nt) in enumerate(calls):
                    idx_sb = ep.tile([P, cfg.TPC * 8], I16, tag="idx")
                    nc.sync.dma_start(out=idx_sb[:, :nt * 8],
                                      in_=gidx_d[:, t0 * 8:(t0 + nt) * 8])
                    g = gb.tile([P, cfg.TPC, H], F32, tag="gath")
                    if l1:
                        src_ap = table[:, :]
                    elif k == CH:
                        src_ap = hw_shard[:, :]
                    else:
                        src_ap = table[k * CHN:(k + 1) * CHN, :]
                    gi = nc.gpsimd.dma_gather(
                        out_ap=g[:, :nt, :], in_ap=src_ap,
                        idxs_ap=idx_sb[:, :nt * 8],
                        num_idxs=nt * P, num_idxs_reg=nt * P,
                        elem_size=H, single_packet=False,
                        queue_num=pd["i"] % 2)
                    chain_pool_dma(gi)
                    gbuf[ci] = g

                # meta blocks of 64 tiles
                MB = 64
                Ttot = T1 if l1 else T23
                mblocks = {}
                for mb0 in range(0, Ttot, MB):
                    mb = ep.tile([P, MB, 2], F32, tag="meta")
                    n = min(MB, Ttot - mb0)
                    nc.sync.dma_start(out=mb[:, :n, :],
                                      in_=meta_d[:, mb0:mb0 + n, :])
                    mblocks[mb0] = mb

                # call lookup: tile t -> (call idx, slot)
                call_of = {}
                for ci, (k, t0, nt) in enumerate(calls):
                    for i in range(nt):
                        call_of[t0 + i] = (ci, i)

                def emit_tile(t, qpsum, qslot, first, last):
                    ci, slot = call_of[t]
                    g = gbuf[ci]
                    mb = mblocks[(t // MB) * MB]
                    tt = t - (t // MB) * MB
                    B = bq.tile([P, P], F32, tag="B")
                    nc.vector.tensor_scalar(
                        out=B[:], in0=iota_f[:, :P],
                        scalar1=mb[:, tt, 0:1],
                        scalar2=mb[:, tt, 1:2],
                        op0=OP.is_equal, op1=OP.mult)
                    nc.tensor.matmul(
                        out=qpsum[:, qslot * P:(qslot + 1) * P],
                        lhsT=g[:, slot, :], rhs=B[:],
                        start=first, stop=last)

                # windows grouped in quads of 4 sharing one PSUM bank;
                # whole quad accumulated, then flushed in contiguous runs
                # of non-empty windows (one op per run).
                t = 0
                for k in (range(1) if l1 else range(CH + 1)):
                    wrow = st.t1_w if l1 else st.t23_kw[k]
                    for q0 in range(0, W, 4):
                        qws = [w for w in range(q0, min(q0 + 4, W))]
                        if all(wrow[w] == 0 for w in qws):
                            continue
                        qpsum = eps.tile([P, 512], F32, tag="qp")
                        for w in qws:
                            Tw = wrow[w]
                            for i in range(Tw):
                                emit_tile(t, qpsum, w - q0, i == 0,
                                          i == Tw - 1)
                                t += 1
                        # flush runs of non-empty windows
                        run = []
                        for w in qws + [None]:
                            if w is not None and wrow[w] > 0:
                                run.append(w)
                                continue
                            if run:
                                s0, s1 = run[0] - q0, run[-1] - q0 + 1
                                sl = qpsum[:, s0 * P:s1 * P]
                                col = run[0] * P
                                ncol = (s1 - s0) * P
                                if l1:
                                    nc.scalar.activation(
                                        out=h_out[:, col:col + ncol],
                                        in_=sl, func=AF.Relu,
                                        bias=b_cols[:, 0:1], scale=1.0)
                                else:
                                    nc.vector.tensor_tensor(
                                        out=h_acc[:, col:col + ncol],
                                        in0=h_acc[:, col:col + ncol],
                                        in1=sl, op=OP.add)
                            run = []

                if not l1:
                    # final bias (+relu) pass, in place
                    func = AF.Relu if layer < cfg.L - 1 else AF.Identity
                    CW = 512
                    for s0 in range(0, NCP, CW):
                        nn = min(CW, NCP - s0)
                        nc.scalar.activation(
                            out=h_acc[:, s0:s0 + nn], in_=h_acc[:, s0:s0 + nn],
                            func=func, bias=b_cols[:, layer:layer + 1],
                            scale=1.0)

        def hw_phase(layer, h_in):
            """Compute hw_shard = (h_in^T @ W_l) node-major and AllGather."""
            with tc.tile_pool(name=f"hw{layer}", bufs=3) as hp, \
                 tc.tile_pool(name=f"hwps{layer}", bufs=2, space="PSUM") as hps, \
                 tc.tile_pool(name=f"hwps2{layer}", bufs=2, space="PSUM") as hps2:
                nc.sync.dma_start(out=w_sb[:], in_=Ws_d[layer])
                CWW = 512
                for j0 in range(0, NC, CWW):
                    nj = min(CWW, NC - j0)
                    ps = hps.tile([P, CWW], F32, tag="mm")
                    nc.tensor.matmul(out=ps[:, :nj], lhsT=w_sb[:],
                                     rhs=h_in[:, j0:j0 + nj],
                                     start=True, stop=True)
                    hw_s = hp.tile([P, CWW], F32, tag="hw_s")
                    nc.scalar.activation(out=hw_s[:, :nj], in_=ps[:, :nj],
                                         func=AF.Copy)
                    for q0 in range(0, nj, P):
                        nq = min(P, nj - q0)
                        pt = hps2.tile([P, P], F32, tag="tr")
                        nc.tensor.transpose(out=pt[:nq, :],
                                            in_=hw_s[:, q0:q0 + nq],
                                            identity=ident[:])
                        stg = hp.tile([P, P], F32, tag="stg")
                        nc.scalar.activation(out=stg[:nq, :], in_=pt[:nq, :],
                                             func=AF.Copy)
                        nc.sync.dma_start(
                            out=hw_shard[j0 + q0:j0 + q0 + nq, :],
                            in_=stg[:nq, :])
            nc.gpsimd.collective_compute(
                "AllGather", OP.bypass, replica_groups=[cores],
                ins=[hw_shard[:, :]], outs=[hw_full[:, :]])

        if stage >= 2:
            with nc.named_scope("layer1"):
                edge_pass(0, hT_a, hT_a)
        if stage >= 3:
            with nc.named_scope("hw2"):
                hw_phase(1, hT_a)
        if stage >= 4:
            with nc.named_scope("layer2"):
                edge_pass(1, hT_b, hT_b)
        if stage >= 5:
            with nc.named_scope("hw3"):
                hw_phase(2, hT_b)
            with nc.named_scope("layer3"):
                edge_pass(2, hT_a, hT_a)
        if stage < 99:
            # placeholder output so the program is complete
            with tc.tile_pool(name="dbg", bufs=1) as dbg:
                d = dbg.tile([P, H], F32)
                if stage >= 2:
                    nc.vector.tensor_copy(out=d[:], in_=hT_a[:, :H])
                else:
                    nc.vector.memset(d[:], 1.0)
                for gw in range(GW):
                    nc.sync.dma_start(out=out_d[gw * P:(gw + 1) * P, :],
                                      in_=d[:])
            return nc

        # ---------------- pooling ----------------
        with nc.named_scope("pool"), \
             tc.tile_pool(name="po", bufs=3) as po, \
             tc.tile_pool(name="po_ps", bufs=2, space="PSUM") as po_ps, \
             tc.tile_pool(name="po_acc", bufs=1, space="PSUM") as po_acc:
            pm = po.tile([P, W], F32, tag="pm")
            nc.sync.dma_start(out=pm[:], in_=pool_meta[:, :])
            gcols = po.tile([P, 2], I32, tag="gcols")
            nc.sync.dma_start(out=gcols[:], in_=gid_cols[:, :])
            recip_sb = po.tile([P, GW], F32, tag="recip")
            nc.sync.dma_start(out=recip_sb[:], in_=recip_pm[:, :])

            acc = po_acc.tile([P, GS], F32)
            for t in range(W):
                # node-major h3 tile
                pt = po_ps.tile([P, P], F32, tag="ptr")
                nc.tensor.transpose(out=pt[:], in_=hT_a[:, t * P:(t + 1) * P],
                                    identity=ident[:])
                h3nm = po.tile([P, P], F32, tag="h3nm")
                nc.scalar.activation(out=h3nm[:], in_=pt[:], func=AF.Copy)
                Bp = po.tile([P, GS], F32, tag="Bp")
                nc.vector.tensor_scalar(
                    out=Bp[:], in0=iota_f[:],
                    scalar1=pm[:, t:t + 1], scalar2=None,
                    op0=OP.is_equal)
                nc.tensor.matmul(out=acc[:], lhsT=h3nm[:], rhs=Bp[:],
                                 start=(t == 0), stop=(t == W - 1))

            def dummy_gather():
                dz = po.tile([P, 1, P], F32, tag="dz")
                zi = po.tile([P, 8], I16, tag="zi")
                nc.vector.memset(zi[:], 0)
                gi = nc.gpsimd.dma_gather(
                    out_ap=dz[:], in_ap=t1_dram[:, :], idxs_ap=zi[:],
                    num_idxs=P, num_idxs_reg=P, elem_size=H,
                    single_packet=False, queue_num=pd["i"] % 2)
                chain_pool_dma(gi)

            # zero pooled_nm
            zt = po.tile([P, P], F32, tag="zt")
            nc.vector.memset(zt[:], 0.0)
            for r0 in range(0, cfg.G + GS, P):
                nc.sync.dma_start(out=pooled_nm[r0:r0 + P, :], in_=zt[:])

            # transpose pooledT [H, GS] back to rows, scatter by graph id
            acc_sb = po.tile([P, GS], F32, tag="acc_sb")
            nc.scalar.activation(out=acc_sb[:], in_=acc[:], func=AF.Copy)
            for half in range(2):
                pt = po_ps.tile([P, P], F32, tag="ptr")
                nc.tensor.transpose(out=pt[:],
                                    in_=acc_sb[:, half * P:(half + 1) * P],
                                    identity=ident[:])
                rows = po.tile([P, P], F32, tag="rows")
                nc.scalar.activation(out=rows[:], in_=pt[:], func=AF.Copy)
                if pd["i"] % 2 == 1:
                    dummy_gather()  # scatters run on queue 0: need even lane
                si = nc.gpsimd.indirect_dma_start(
                    out=pooled_nm[:, :],
                    out_offset=IndirectOffsetOnAxis(
                        ap=gcols[:, half:half + 1], axis=0),
                    in_=rows[:], in_offset=None)
                chain_pool_dma(si)

            nc.gpsimd.collective_compute(
                "AllReduce", OP.add, replica_groups=[cores],
                ins=[pooled_nm[:, :]], outs=[pooled_sum[:, :]])

            for gw in range(GW):
                ot = po.tile([P, H], F32, tag="ot")
                nc.sync.dma_start(out=ot[:],
                                  in_=pooled_sum[gw * P:(gw + 1) * P, :])
                os = po.tile([P, H], F32, tag="os")
                nc.vector.tensor_scalar(
                    out=os[:], in0=ot[:], scalar1=recip_sb[:, gw:gw + 1],
                    scalar2=None, op0=OP.mult)
                nc.sync.dma_start(out=out_d[gw * P:(gw + 1) * P, :],
                                  in_=os[:])

    return nc


# --------------------------------------------------------------------------
# entry point: full inputs -> full output
# --------------------------------------------------------------------------

_CACHE = {}


def _get_compiled(cfg, st_key, st):
    if st_key not in _CACHE:
        nc = build_nc(cfg, st)
        nc.finalize()
        _CACHE[st_key] = nc
    return _CACHE[st_key]


def kernel(x, edge_index, batch, emb_table, Ws, bs):
    cfg = Cfg()  # full problem size, hardcoded
    st, in_maps, _ = preprocess(x, edge_index, batch, emb_table, Ws, bs, cfg)
    st_key = (tuple(st.t1_w), tuple(tuple(r) for r in st.t23_kw))
    nc = _get_compiled(cfg, st_key, st)

    from concourse.bass_utils import run_bass_kernel_spmd

    res = run_bass_kernel_spmd(nc, in_maps, list(range(cfg.C)))
    return np.ascontiguousarray(res.results[0]["out"])



# revision 7
# speedup vs baseline: 1.5280x; 1.5280x over previous
"""GCN embedder kernel for TRN2, 8-core SPMD (v2, bf16 + 4 SWDGE queues).

Design
------
* Nodes sharded contiguously across C=8 cores (NC nodes each). Edges
  (incl. self-loops) are owned by the dst core.
* Node features h are kept feature-major in SBUF as bf16: hT [H=128, NCP].
* Gather tables (T1 = emb@W1 for layer 1, hw_full = h@W_l for layers 2/3)
  are bf16 in DRAM; dma_gather cost is purely per-index (~2ns/idx with 4
  SWDGE queues), so bf16 halves SBUF/DRAM pressure at no gather cost.
* Edge pass per layer, quad-major: dst windows of 128 nodes grouped in
  quads of 4 (one PSUM bank [128, 512] per quad). For each quad, tiles
  from ALL source chunks (CH=4 table chunks + 1 local self chunk)
  accumulate into the quad PSUM; a single fused ACT (bias + relu) flushes
  PSUM -> hT bf16. No SBUF accumulator, no memset, no extra bias pass.
* Per 128-edge tile: one DVE tensor_scalar builds the norm-scaled one-hot
  B[e, d] = (iota[d] == dstrel[e]) * norm[e] in bf16 (~226ns), and one PE
  matmul (lhsT=M gathered bf16, rhs=B) accumulates [H, 128] into the
  window slot of the quad PSUM.
* Gather calls are per (quad, chunk) for layers 2/3 and per-quad 32-tile
  chops for layer 1, chained in issue order on SWDGE queues i%4 (queue
  index must match the tile framework's DMASW lane round-robin).
* Pooling: transpose h3 windows to node-major bf16; indicator matmul
  against batchrel one-hot (bf16) accumulates pooledT [H, 256] in PSUM;
  transpose back to f32 rows, scatter by graph id (indirect DMA, queue
  parity aligned with dummy gathers); AllReduce f32; multiply by 1/cnt.

All structure (tile counts per window, call sizes) is maxed across cores
so the single SPMD program fits every core; pad slots have norm=0 (B
column is zero) and index 0 (valid row).
"""

import math
from contextlib import ExitStack
from dataclasses import dataclass, field

import numpy as np

import concourse.mybir as mybir
import concourse.tile as tile
from concourse import bacc, bass
from concourse.bass import AP, IndirectOffsetOnAxis, ds
from concourse.masks import make_identity

F32 = mybir.dt.float32
BF16 = mybir.dt.bfloat16
I16 = mybir.dt.int16
I32 = mybir.dt.int32
AF = mybir.ActivationFunctionType
OP = mybir.AluOpType

P = 128  # partitions / hidden size / vocab


@dataclass
class Cfg:
    N: int = 100000
    E: int = 1600000
    H: int = 128
    V: int = 128
    L: int = 3
    G: int = 1024
    C: int = 8          # cores
    CH: int = 4         # gather-table chunks (int16 index limit)
    TPC: int = 32       # max tiles per layer-1 dma_gather call
    NQ: int = 4         # SWDGE queues

    @property
    def NC(self):
        assert self.N % self.C == 0
        return self.N // self.C

    @property
    def CHN(self):
        assert self.N % self.CH == 0
        return self.N // self.CH

    @property
    def W(self):  # dst windows per core
        return math.ceil(self.NC / P)

    @property
    def Q(self):  # window quads per core
        return math.ceil(self.W / 4)

    @property
    def NCP(self):
        return self.W * P

    @property
    def GSPAN(self):
        return 256


@dataclass
class Structure:
    t1_w: list = field(default_factory=list)        # [W] tiles per window, l1
    t23_kw: list = field(default_factory=list)      # [CH+1][W]
    calls1: list = field(default_factory=list)      # [(q, t0, nt)]
    calls23: list = field(default_factory=list)     # [(q, k, t0, nt)]

    @property
    def T1(self):
        return sum(self.t1_w)

    @property
    def T23(self):
        return sum(sum(r) for r in self.t23_kw)


def preprocess(x, edge_index, batch, emb_table, Ws, bs, cfg: Cfg):
    """Host-side (index-only) preprocessing."""
    N, E, C, CH = cfg.N, cfg.E, cfg.C, cfg.CH
    NC, CHN, W, Q = cfg.NC, cfg.CHN, cfg.W, cfg.Q
    CHX = CH + 1

    x = np.asarray(x).astype(np.int64)
    edge_index = np.asarray(edge_index).astype(np.int64)
    batch = np.asarray(batch).astype(np.int64)

    loop = np.arange(N, dtype=np.int64)
    src = np.concatenate([edge_index[0], loop])
    dst = np.concatenate([edge_index[1], loop])
    deg = np.bincount(dst, minlength=N).astype(np.float32)
    dinv = 1.0 / np.sqrt(deg)  # deg >= 1 thanks to self loops
    norm = (dinv[src] * dinv[dst]).astype(np.float32)
    xsrc = x[src]
    dinv2 = (dinv * dinv).astype(np.float32)

    owner = dst // NC

    per_core = []
    for c in range(C):
        m = owner == c
        d_c = dst[m] - c * NC
        # layer-1 stream: all edges+loops sorted by dst (quad-major implied)
        o1 = np.argsort(d_c, kind="stable")
        # layer-2/3 stream: edges sorted by (quad, chunk, dst); self edges
        # are a synthetic chunk k=CH gathered from the local hw_shard.
        m23 = owner[:E] == c
        s23 = src[:E][m23]
        d23 = dst[:E][m23] - c * NC
        n23 = norm[:E][m23]
        ck23 = s23 // CHN
        vloc = np.arange(NC, dtype=np.int64)
        s23 = np.concatenate([s23, vloc])
        d23 = np.concatenate([d23, vloc])
        n23 = np.concatenate([n23, dinv2[c * NC + vloc]])
        ck23 = np.concatenate([ck23, np.full(NC, CH, np.int64)])
        q23 = d23 // 512
        o23 = np.lexsort((d23, ck23, q23))
        srel23 = np.where(ck23 == CH, s23, s23 - ck23 * CHN)
        per_core.append(dict(
            s=src[m], d=d_c, n=norm[m], xs=xsrc[m], o1=o1,
            s23=srel23, d23=d23, n23=n23, ck23=ck23, o23=o23))

    # ---- uniform tile counts (maxed across cores) ----
    t1_w = np.zeros(W, dtype=np.int64)
    t23_kw = np.zeros((CHX, W), dtype=np.int64)
    for c in range(C):
        pc = per_core[c]
        w1 = pc["d"][pc["o1"]] // P
        cnt1 = np.bincount(w1, minlength=W)
        t1_w = np.maximum(t1_w, -(-cnt1 // P))
        dk = pc["d23"][pc["o23"]]
        kk = pc["ck23"][pc["o23"]]
        gid = kk * W + dk // P
        cntk = np.bincount(gid, minlength=CHX * W).reshape(CHX, W)
        t23_kw = np.maximum(t23_kw, -(-cntk // P))
    assert (t1_w >= 1).all()
    assert (t23_kw[CH] == 1).all()

    st = Structure(t1_w=list(t1_w), t23_kw=[list(r) for r in t23_kw])

    # ---- call lists ----
    # layer 1: per-quad 32-tile chops, stream order = (q, w, tile)
    calls1 = []
    toff = 0
    for q in range(Q):
        ws = range(q * 4, min(q * 4 + 4, W))
        tq = int(sum(t1_w[w] for w in ws))
        t = 0
        while t < tq:
            nt = min(cfg.TPC, tq - t)
            calls1.append((q, toff + t, nt))
            t += nt
        toff += tq
    st.calls1 = calls1
    # layers 2/3: one call per (q, k), stream order = (q, k, w, tile)
    calls23 = []
    toff = 0
    for q in range(Q):
        ws = range(q * 4, min(q * 4 + 4, W))
        for k in range(CHX):
            tqk = int(sum(t23_kw[k][w] for w in ws))
            assert tqk >= 1
            calls23.append((q, k, toff, tqk))
            toff += tqk
    st.calls23 = calls23

    # ---- build padded per-core streams ----
    def build_stream(d_sorted, n_sorted, idx_sorted, group_of_edge,
                     counts_T, n_groups):
        """Pack a sorted edge stream into per-group padded tiles.

        group_of_edge: group ordinal per edge (sorted ascending)
        counts_T: tiles per group ordinal
        returns meta [P, Ttot, 2] f32 (dstrel%128, norm), idxs [Ttot*128] i16
        """
        Ttot = int(sum(counts_T))
        meta = np.zeros((P, Ttot, 2), dtype=np.float32)
        idxs = np.zeros(Ttot * P, dtype=np.int16)
        cnt = np.bincount(group_of_edge, minlength=n_groups)
        starts = np.concatenate([[0], np.cumsum(cnt)[:-1]])
        t0 = 0
        for g in range(n_groups):
            cg, sg, Tg = int(cnt[g]), int(starts[g]), int(counts_T[g])
            assert cg <= Tg * P, (g, cg, Tg)
            sl = slice(sg, sg + cg)
            ii = np.arange(cg)
            tt = t0 + ii // P
            pp = ii % P
            meta[pp, tt, 0] = (d_sorted[sl] % P).astype(np.float32)
            meta[pp, tt, 1] = n_sorted[sl]
            idxs[t0 * P + ii] = idx_sorted[sl].astype(np.int16)
            t0 += Tg
        return meta, idxs

    def wrap_idxs(idx_stream, calls, nidx_cols):
        out = np.zeros((P, nidx_cols), dtype=np.int16)
        col = 0
        for (t0, nt) in calls:
            arr = idx_stream[t0 * P:(t0 + nt) * P]
            wr = arr.reshape(-1, 16).T  # [16, nt*8]
            out[:, col:col + nt * 8] = np.tile(wr, (8, 1))
            col += nt * 8
        assert col == nidx_cols
        return out

    T1, T23 = st.T1, st.T23

    # group ordinal maps (stream order)
    # l1: ordinal of window w = w (stream sorted by dst => window-major,
    # and windows are quad-contiguous)
    # l23: ordinal of (q, k, w) in stream order
    ord23 = np.full((CHX, W), -1, dtype=np.int64)
    counts23 = []
    o = 0
    for q in range(Q):
        ws = range(q * 4, min(q * 4 + 4, W))
        for k in range(CHX):
            for w in ws:
                ord23[k, w] = o
                counts23.append(int(t23_kw[k][w]))
                o += 1
    n_groups23 = o

    in_maps = []
    for c in range(C):
        pc = per_core[c]
        o1 = pc["o1"]
        d1, n1, xs1 = pc["d"][o1], pc["n"][o1], pc["xs"][o1]
        w1 = d1 // P
        meta1, idx1 = build_stream(d1, n1, xs1, w1, list(t1_w), W)

        o23 = pc["o23"]
        dk, nk, srel, kk = (pc["d23"][o23], pc["n23"][o23],
                            pc["s23"][o23], pc["ck23"][o23])
        g23 = ord23[kk, dk // P]
        assert (np.diff(g23) >= 0).all()
        meta23, idx23 = build_stream(dk, nk, srel, g23, counts23, n_groups23)

        gidx1 = wrap_idxs(idx1, [(t0, nt) for (_q, t0, nt) in st.calls1],
                          T1 * 8)
        gidx23 = wrap_idxs(idx23, [(t0, nt) for (_q, _k, t0, nt)
                                   in st.calls23], T23 * 8)

        # pooling metadata
        nodes = np.arange(cfg.NCP) + c * NC
        valid = nodes < (c + 1) * NC
        bvals = np.where(valid, batch[np.minimum(nodes, N - 1)], -1)
        gmin = int(batch[c * NC])
        gmax = int(batch[min((c + 1) * NC, N) - 1])
        assert gmax - gmin < cfg.GSPAN, (c, gmin, gmax)
        brel = np.where(valid, bvals - gmin, -1).astype(np.float32)
        pool_meta = brel.reshape(cfg.W, P).T.copy()  # [128, W]
        gid_rows = gmin + np.arange(cfg.GSPAN)
        gid_rows = np.where(gid_rows < cfg.G, gid_rows,
                            cfg.G + np.arange(cfg.GSPAN) % 256).astype(np.int32)
        gid_cols = gid_rows.reshape(2, P).T.copy()  # [128, 2]

        cnts = np.bincount(batch, minlength=cfg.G).astype(np.float32)
        recip = 1.0 / np.maximum(cnts, 1.0)
        recip_pm = recip.reshape(cfg.G // P, P).T.copy()

        in_maps.append({
            "meta1": meta1, "gidx1": gidx1,
            "meta23": meta23, "gidx23": gidx23,
            "pool_meta": pool_meta, "gid_cols": gid_cols,
            "recip_pm": recip_pm,
            "emb": np.asarray(emb_table, dtype=np.float32),
            "Ws": np.asarray(Ws, dtype=np.float32),
            "bs": np.asarray(bs, dtype=np.float32),
        })

    return st, in_maps


# --------------------------------------------------------------------------
# device program
# --------------------------------------------------------------------------

DEBUG_STAGE = 0  # 0=off; 1=dump hT after layer1; 2=after layer2; 3=after layer3


def build_nc(cfg: Cfg, st: Structure):
    N, H, C, CH, W, Q = cfg.N, cfg.H, cfg.C, cfg.CH, cfg.W, cfg.Q
    NC, CHN, NCP = cfg.NC, cfg.CHN, cfg.NCP
    CHX = CH + 1
    T1, T23 = st.T1, st.T23
    GS = cfg.GSPAN
    GW = cfg.G // P
    NQ = cfg.NQ
    TQK = max(nt for (_q, _k, _t0, nt) in st.calls23)

    nc = bacc.Bacc(None, num_devices=C, num_swdge_queues=NQ)
    cores = list(range(C))

    # ---- external I/O ----
    meta1 = nc.declare_dram_parameter("meta1", [P, T1, 2], F32, isOutput=False)
    gidx1 = nc.declare_dram_parameter("gidx1", [P, T1 * 8], I16, isOutput=False)
    meta23 = nc.declare_dram_parameter("meta23", [P, T23, 2], F32, isOutput=False)
    gidx23 = nc.declare_dram_parameter("gidx23", [P, T23 * 8], I16, isOutput=False)
    pool_meta = nc.declare_dram_parameter("pool_meta", [P, W], F32, isOutput=False)
    gid_cols = nc.declare_dram_parameter("gid_cols", [P, 2], I32, isOutput=False)
    recip_pm = nc.declare_dram_parameter("recip_pm", [P, GW], F32, isOutput=False)
    emb_d = nc.declare_dram_parameter("emb", [P, H], F32, isOutput=False)
    Ws_d = nc.declare_dram_parameter("Ws", [cfg.L, H, H], F32, isOutput=False)
    bs_d = nc.declare_dram_parameter("bs", [cfg.L, H], F32, isOutput=False)
    out_d = nc.declare_dram_parameter("out", [cfg.G, H], F32, isOutput=True)

    # ---- internal DRAM ----
    t1_dram = nc.dram_tensor("t1_tab", [cfg.V, H], BF16)
    hw_shard = nc.dram_tensor("hw_shard", [NC, H], BF16)
    hw_full = nc.dram_tensor("hw_full", [N, H], BF16, addr_space="Shared")
    pooled_nm = nc.dram_tensor("pooled_nm", [cfg.G + GS, H], F32)
    pooled_sum = nc.dram_tensor("pooled_sum", [cfg.G + GS, H], F32,
                                addr_space="Shared")

    from concourse.tile import add_dep_helper
    pd = {"i": 0, "last": None}

    def chain_pool_dma(inst):
        if pd["last"] is not None:
            add_dep_helper(inst.ins, pd["last"].ins, sync=False,
                           reason="pool-dma queue/lane parity order")
        pd["last"] = inst
        pd["i"] += 1

    with tile.TileContext(nc) as tc, ExitStack() as ctx:
        const = ctx.enter_context(tc.tile_pool(name="const", bufs=1))
        hpool = ctx.enter_context(tc.tile_pool(name="hbuf", bufs=1))

        ident = const.tile([P, P], F32)
        make_identity(nc, ident[:])
        ident_bf = const.tile([P, P], BF16)
        make_identity(nc, ident_bf[:])
        iota_i = const.tile([P, GS], I32)
        nc.gpsimd.iota(iota_i[:], pattern=[[1, GS]], base=0,
                       channel_multiplier=0)
        iota_bf = const.tile([P, GS], BF16)
        nc.vector.tensor_copy(out=iota_bf[:], in_=iota_i[:])

        w_sb = const.tile([P, H], F32, tag="w_sb")
        w_bf = const.tile([P, H], BF16, tag="w_bf")
        emb_sb = const.tile([P, H], F32, tag="w_sb2")
        b_cols = const.tile([P, cfg.L], F32)
        for l in range(cfg.L):
            nc.sync.dma_start(out=b_cols[:, l:l + 1], in_=bs_d[l, :, None])

        hT_a = hpool.tile([P, NCP], BF16)   # layer outputs (ping)
        hT_b = hpool.tile([P, NCP], BF16)   # (pong)

        # ---------------- T1 = emb @ W1 (bf16 table) ----------------
        with tc.tile_pool(name="pro", bufs=2) as pro, \
             tc.tile_pool(name="pro_ps", bufs=2, space="PSUM") as pro_ps:
            nc.sync.dma_start(out=emb_sb[:], in_=emb_d[:, :])
            nc.sync.dma_start(out=w_sb[:], in_=Ws_d[0])
            embT_ps = pro_ps.tile([P, P], F32)
            nc.tensor.transpose(out=embT_ps[:], in_=emb_sb[:], identity=ident[:])
            embT = pro.tile([P, P], F32)
            nc.vector.tensor_copy(out=embT[:], in_=embT_ps[:])
            t1t_ps = pro_ps.tile([P, P], F32)
            nc.tensor.matmul(out=t1t_ps[:], lhsT=w_sb[:], rhs=embT[:],
                             start=True, stop=True)
            t1t = pro.tile([P, P], F32)
            nc.vector.tensor_copy(out=t1t[:], in_=t1t_ps[:])
            t1nm_ps = pro_ps.tile([P, P], F32)
            nc.tensor.transpose(out=t1nm_ps[:], in_=t1t[:], identity=ident[:])
            t1nm = pro.tile([P, P], BF16)
            nc.vector.tensor_copy(out=t1nm[:], in_=t1nm_ps[:])
            nc.sync.dma_start(out=t1_dram[:, :], in_=t1nm[:])

        # ---------------- edge pass ----------------
        def edge_pass(layer, h_out):
            l1 = layer == 0
            meta_d, gidx_d = (meta1, gidx1) if l1 else (meta23, gidx23)
            calls = ([(q, None, t0, nt) for (q, t0, nt) in st.calls1] if l1
                     else st.calls23)
            MAXT = cfg.TPC if l1 else TQK

            with tc.tile_pool(name=f"ep{layer}", bufs=8) as ep, \
                 tc.tile_pool(name=f"gb{layer}", bufs=6) as gb, \
                 tc.tile_pool(name=f"bq{layer}", bufs=4) as bq, \
                 tc.tile_pool(name=f"eps{layer}", bufs=2, space="PSUM") as eps:

                # issue all gathers + idx/meta loads, keyed by call index
                gbuf, mbuf = {}, {}
                idxcol = 0
                for ci, (q, k, t0, nt) in enumerate(calls):
                    idx_sb = ep.tile([P, MAXT * 8], I16, tag="idx")
                    nc.sync.dma_start(out=idx_sb[:, :nt * 8],
                                      in_=gidx_d[:, idxcol:idxcol + nt * 8])
                    idxcol += nt * 8
                    mb = ep.tile([P, MAXT, 2], F32, tag="meta")
                    nc.sync.dma_start(out=mb[:, :nt, :],
                                      in_=meta_d[:, t0:t0 + nt, :])
                    g = gb.tile([P, MAXT, H], BF16, tag="gath")
                    if l1:
                        src_ap = t1_dram[:, :]
                    elif k == CH:
                        src_ap = hw_shard[:, :]
                    else:
                        src_ap = hw_full[k * CHN:(k + 1) * CHN, :]
                    gi = nc.gpsimd.dma_gather(
                        out_ap=g[:, :nt, :], in_ap=src_ap,
                        idxs_ap=idx_sb[:, :nt * 8],
                        num_idxs=nt * P, num_idxs_reg=nt * P,
                        elem_size=H, single_packet=False,
                        queue_num=pd["i"] % NQ)
                    chain_pool_dma(gi)
                    gbuf[ci] = g
                    mbuf[ci] = mb

                # per-tile emit, walking quads
                def emit_tile(ci, slot, qpsum, wl, first, last):
                    g, mb = gbuf[ci], mbuf[ci]
                    B = bq.tile([P, P], BF16, tag="B")
                    nc.vector.tensor_scalar(
                        out=B[:], in0=iota_bf[:, :P],
                        scalar1=mb[:, slot, 0:1],
                        scalar2=mb[:, slot, 1:2],
                        op0=OP.is_equal, op1=OP.mult)
                    nc.tensor.matmul(
                        out=qpsum[:, wl * P:(wl + 1) * P],
                        lhsT=g[:, slot, :], rhs=B[:],
                        start=first, stop=last)

                # call index lookup per (q[,k])
                ci_of = {}
                for ci, (q, k, t0, nt) in enumerate(calls):
                    ci_of.setdefault((q, k), []).append(ci)

                func = AF.Relu if layer < cfg.L - 1 else AF.Identity
                for q in range(Q):
                    ws = list(range(q * 4, min(q * 4 + 4, W)))
                    qpsum = eps.tile([P, 512], F32, tag="qp")
                    if l1:
                        # stream order (q, w, tile); chopped calls
                        cis = ci_of[(q, None)]
                        pos = 0  # position within quad tile stream
                        sizes = [calls[ci][3] for ci in cis]
                        for w in ws:
                            Tw = st.t1_w[w]
                            for i in range(Tw):
                                acc, rem = 0, pos
                                for ci, sz in zip(cis, sizes):
                                    if rem < sz:
                                        break
                                    rem -= sz
                                emit_tile(ci, rem, qpsum, w - q * 4,
                                          i == 0, i == Tw - 1)
                                pos += 1
                    else:
                        # window-major so each window's PSUM accumulation
                        # group is contiguous (a later start=True must not
                        # interleave with other windows' accumulation)
                        for w in ws:
                            tot = sum(st.t23_kw[k][w] for k in range(CHX))
                            done = 0
                            for k in range(CHX):
                                (ci,) = ci_of[(q, k)]
                                slot = sum(st.t23_kw[k][w2]
                                           for w2 in ws if w2 < w)
                                for i in range(st.t23_kw[k][w]):
                                    emit_tile(ci, slot + i, qpsum, w - q * 4,
                                              done == 0, done == tot - 1)
                                    done += 1
                    ncol = len(ws) * P
                    nc.scalar.activation(
                        out=h_out[:, q * 512:q * 512 + ncol],
                        in_=qpsum[:, :ncol], func=func,
                        bias=b_cols[:, layer:layer + 1], scale=1.0)

        # ---------------- hw phase ----------------
        def hw_phase(layer, h_in):
            with tc.tile_pool(name=f"hw{layer}", bufs=3) as hp, \
                 tc.tile_pool(name=f"hwps{layer}", bufs=2, space="PSUM") as hps, \
                 tc.tile_pool(name=f"hwps2{layer}", bufs=2, space="PSUM") as hps2:
                nc.sync.dma_start(out=w_sb[:], in_=Ws_d[layer])
                nc.vector.tensor_copy(out=w_bf[:], in_=w_sb[:])
                CWW = 512
                for j0 in range(0, NC, CWW):
                    nj = min(CWW, NC - j0)
                    ps = hps.tile([P, CWW], F32, tag="mm")
                    nc.tensor.matmul(out=ps[:, :nj], lhsT=w_bf[:],
                                     rhs=h_in[:, j0:j0 + nj],
                                     start=True, stop=True)
                    hw_s = hp.tile([P, CWW], F32, tag="hw_s")
                    nc.scalar.activation(out=hw_s[:, :nj], in_=ps[:, :nj],
                                         func=AF.Copy)
                    for q0 in range(0, nj, P):
                        nq = min(P, nj - q0)
                        pt = hps2.tile([P, P], F32, tag="tr")
                        nc.tensor.transpose(out=pt[:nq, :],
                                            in_=hw_s[:, q0:q0 + nq],
                                            identity=ident[:])
                        stg = hp.tile([P, P], BF16, tag="stg")
                        nc.scalar.activation(out=stg[:nq, :], in_=pt[:nq, :],
                                             func=AF.Copy)
                        nc.sync.dma_start(
                            out=hw_shard[j0 + q0:j0 + q0 + nq, :],
                            in_=stg[:nq, :])
            nc.gpsimd.collective_compute(
                "AllGather", OP.bypass, replica_groups=[cores],
                ins=[hw_shard[:, :]], outs=[hw_full[:, :]])

        def dump_h(src_tile):
            dbg_d = nc.declare_dram_parameter("dbg", [P, NCP], F32,
                                              isOutput=True)
            with tc.tile_pool(name="dbg", bufs=2) as dbp:
                CWD = 512
                for s0 in range(0, NCP, CWD):
                    nn = min(CWD, NCP - s0)
                    dt_ = dbp.tile([P, CWD], F32, tag="d")
                    nc.vector.tensor_copy(out=dt_[:, :nn],
                                          in_=src_tile[:, s0:s0 + nn])
                    nc.sync.dma_start(out=dbg_d[:, s0:s0 + nn],
                                      in_=dt_[:, :nn])

        with nc.named_scope("layer1"):
            edge_pass(0, hT_a)
        if DEBUG_STAGE == 1:
            dump_h(hT_a)
        with nc.named_scope("hw2"):
            hw_phase(1, hT_a)
        with nc.named_scope("layer2"):
            edge_pass(1, hT_b)
        if DEBUG_STAGE == 2:
            dump_h(hT_b)
        with nc.named_scope("hw3"):
            hw_phase(2, hT_b)
        with nc.named_scope("layer3"):
            edge_pass(2, hT_a)
        if DEBUG_STAGE == 3:
            dump_h(hT_a)

        # ---------------- pooling ----------------
        with nc.named_scope("pool"), \
             tc.tile_pool(name="po", bufs=3) as po, \
             tc.tile_pool(name="po_ps", bufs=2, space="PSUM") as po_ps, \
             tc.tile_pool(name="po_acc", bufs=1, space="PSUM") as po_acc:
            pm = po.tile([P, W], F32, tag="pm")
            nc.sync.dma_start(out=pm[:], in_=pool_meta[:, :])
            gcols = po.tile([P, 2], I32, tag="gcols")
            nc.sync.dma_start(out=gcols[:], in_=gid_cols[:, :])
            recip_sb = po.tile([P, GW], F32, tag="recip")
            nc.sync.dma_start(out=recip_sb[:], in_=recip_pm[:, :])

            acc = po_acc.tile([P, GS], F32)
            for t in range(W):
                pt = po_ps.tile([P, P], BF16, tag="ptr")
                nc.tensor.transpose(out=pt[:], in_=hT_a[:, t * P:(t + 1) * P],
                                    identity=ident_bf[:])
                h3nm = po.tile([P, P], BF16, tag="h3nm")
                nc.scalar.activation(out=h3nm[:], in_=pt[:], func=AF.Copy)
                Bp = po.tile([P, GS], BF16, tag="Bp")
                nc.vector.tensor_scalar(
                    out=Bp[:], in0=iota_bf[:],
                    scalar1=pm[:, t:t + 1], scalar2=None,
                    op0=OP.is_equal)
                nc.tensor.matmul(out=acc[:], lhsT=h3nm[:], rhs=Bp[:],
                                 start=(t == 0), stop=(t == W - 1))

            def dummy_gather():
                dz = po.tile([P, 1, P], BF16, tag="dz")
                zi = po.tile([P, 8], I16, tag="zi")
                nc.vector.memset(zi[:], 0)
                gi = nc.gpsimd.dma_gather(
                    out_ap=dz[:], in_ap=t1_dram[:, :], idxs_ap=zi[:],
                    num_idxs=P, num_idxs_reg=P, elem_size=H,
                    single_packet=False, queue_num=pd["i"] % NQ)
                chain_pool_dma(gi)

            # zero pooled_nm
            zt = po.tile([P, P], F32, tag="zt")
            nc.vector.memset(zt[:], 0.0)
            for r0 in range(0, cfg.G + GS, P):
                nc.sync.dma_start(out=pooled_nm[r0:r0 + P, :], in_=zt[:])

            acc_sb = po.tile([P, GS], F32, tag="acc_sb")
            nc.scalar.activation(out=acc_sb[:], in_=acc[:], func=AF.Copy)
            for half in range(2):
                pt = po_ps.tile([P, P], F32, tag="ptr")
                nc.tensor.transpose(out=pt[:],
                                    in_=acc_sb[:, half * P:(half + 1) * P],
                                    identity=ident[:])
                rows = po.tile([P, P], F32, tag="rows")
                nc.scalar.activation(out=rows[:], in_=pt[:], func=AF.Copy)
                while pd["i"] % NQ != 0:
                    dummy_gather()  # scatters run on queue 0: align lane
                si = nc.gpsimd.indirect_dma_start(
                    out=pooled_nm[:, :],
                    out_offset=IndirectOffsetOnAxis(
                        ap=gcols[:, half:half + 1], axis=0),
                    in_=rows[:], in_offset=None)
                chain_pool_dma(si)

            nc.gpsimd.collective_compute(
                "AllReduce", OP.add, replica_groups=[cores],
                ins=[pooled_nm[:, :]], outs=[pooled_sum[:, :]])

            for gw in range(GW):
                ot = po.tile([P, H], F32, tag="ot")
                nc.sync.dma_start(out=ot[:],
                                  in_=pooled_sum[gw * P:(gw + 1) * P, :])
                os = po.tile([P, H], F32, tag="os")
                nc.vector.tensor_scalar(
                    out=os[:], in0=ot[:], scalar1=recip_sb[:, gw:gw + 1],
                    scalar2=None, op0=OP.mult)
                nc.sync.dma_start(out=out_d[gw * P:(gw + 1) * P, :],
                                  in_=os[:])

    return nc


# --------------------------------------------------------------------------
# entry point: full inputs -> full output
# --------------------------------------------------------------------------

_CACHE = {}


def _get_compiled(cfg, st_key, st):
    if st_key not in _CACHE:
        nc = build_nc(cfg, st)
        nc.finalize()
        _CACHE[st_key] = nc
    return _CACHE[st_key]


def kernel(x, edge_index, batch, emb_table, Ws, bs):
    cfg = Cfg()  # full problem size, hardcoded
    st, in_maps = preprocess(x, edge_index, batch, emb_table, Ws, bs, cfg)
    st_key = (tuple(st.t1_w), tuple(tuple(r) for r in st.t23_kw))
    nc = _get_compiled(cfg, st_key, st)

    from concourse.bass_utils import run_bass_kernel_spmd

    res = run_bass_kernel_spmd(nc, in_maps, list(range(cfg.C)))
    return np.ascontiguousarray(res.results[0]["out"])


# revision 9
# speedup vs baseline: 1.8085x; 1.1836x over previous
"""GCN embedder kernel for TRN2, 8-core SPMD (v2, bf16 + 4 SWDGE queues).

Design
------
* Nodes sharded contiguously across C=8 cores (NC nodes each). Edges
  (incl. self-loops) are owned by the dst core.
* Node features h are kept feature-major in SBUF as bf16: hT [H=128, NCP].
* Gather tables (T1 = emb@W1 for layer 1, hw_full = h@W_l for layers 2/3)
  are bf16 in DRAM; dma_gather cost is purely per-index (~2ns/idx with 4
  SWDGE queues), so bf16 halves SBUF/DRAM pressure at no gather cost.
* Edge pass per layer, quad-major: dst windows of 128 nodes grouped in
  quads of 4 (one PSUM bank [128, 512] per quad). For each quad, tiles
  from ALL source chunks (CH=4 table chunks + 1 local self chunk)
  accumulate into the quad PSUM; a single fused ACT (bias + relu) flushes
  PSUM -> hT bf16. No SBUF accumulator, no memset, no extra bias pass.
* Per 128-edge tile: one DVE tensor_scalar builds the norm-scaled one-hot
  B[e, d] = (iota[d] == dstrel[e]) * norm[e] in bf16 (~226ns), and one PE
  matmul (lhsT=M gathered bf16, rhs=B) accumulates [H, 128] into the
  window slot of the quad PSUM.
* Gather calls are per (quad, chunk) for layers 2/3 and per-quad 32-tile
  chops for layer 1, chained in issue order on SWDGE queues i%4 (queue
  index must match the tile framework's DMASW lane round-robin).
* Pooling: transpose h3 windows to node-major bf16; indicator matmul
  against batchrel one-hot (bf16) accumulates pooledT [H, 256] in PSUM;
  transpose back to f32 rows, scatter by graph id (indirect DMA, queue
  parity aligned with dummy gathers); AllReduce f32; multiply by 1/cnt.

All structure (tile counts per window, call sizes) is maxed across cores
so the single SPMD program fits every core; pad slots have norm=0 (B
column is zero) and index 0 (valid row).
"""

import math
from contextlib import ExitStack
from dataclasses import dataclass, field

import numpy as np

import concourse.mybir as mybir
import concourse.tile as tile
from concourse import bacc, bass
from concourse.bass import AP, IndirectOffsetOnAxis, ds
from concourse.masks import make_identity

F32 = mybir.dt.float32
BF16 = mybir.dt.bfloat16
I16 = mybir.dt.int16
I32 = mybir.dt.int32
AF = mybir.ActivationFunctionType
OP = mybir.AluOpType

P = 128  # partitions / hidden size / vocab


@dataclass
class Cfg:
    N: int = 100000
    E: int = 1600000
    H: int = 128
    V: int = 128
    L: int = 3
    G: int = 1024
    C: int = 8          # cores
    CH: int = 4         # gather-table chunks (int16 index limit)
    TPC: int = 32       # max tiles per layer-1 dma_gather call
    NQ: int = 4         # SWDGE queues

    @property
    def NC(self):
        assert self.N % self.C == 0
        return self.N // self.C

    @property
    def CHN(self):
        assert self.N % self.CH == 0
        return self.N // self.CH

    @property
    def W(self):  # dst windows per core
        return math.ceil(self.NC / P)

    @property
    def Q(self):  # window quads per core
        return math.ceil(self.W / 4)

    @property
    def NCP(self):
        return self.W * P

    @property
    def GSPAN(self):
        return 256


@dataclass
class Structure:
    t1_q: list = field(default_factory=list)        # [Q] tiles per quad, l1
    t23_qk: list = field(default_factory=list)      # [Q][CH+1]
    calls1: list = field(default_factory=list)      # [(q, t0, nt)]
    calls23: list = field(default_factory=list)     # [(q, k, t0, nt)]

    @property
    def T1(self):
        return sum(self.t1_q)

    @property
    def T23(self):
        return sum(sum(r) for r in self.t23_qk)


def preprocess(x, edge_index, batch, emb_table, Ws, bs, cfg: Cfg):
    """Host-side (index-only) preprocessing."""
    N, E, C, CH = cfg.N, cfg.E, cfg.C, cfg.CH
    NC, CHN, W, Q = cfg.NC, cfg.CHN, cfg.W, cfg.Q
    CHX = CH + 1

    x = np.asarray(x).astype(np.int64)
    edge_index = np.asarray(edge_index).astype(np.int64)
    batch = np.asarray(batch).astype(np.int64)

    loop = np.arange(N, dtype=np.int64)
    src = np.concatenate([edge_index[0], loop])
    dst = np.concatenate([edge_index[1], loop])
    deg = np.bincount(dst, minlength=N).astype(np.float32)
    dinv = 1.0 / np.sqrt(deg)  # deg >= 1 thanks to self loops
    norm = (dinv[src] * dinv[dst]).astype(np.float32)
    xsrc = x[src]
    dinv2 = (dinv * dinv).astype(np.float32)

    owner = dst // NC

    per_core = []
    for c in range(C):
        m = owner == c
        d_c = dst[m] - c * NC
        # layer-1 stream: all edges+loops sorted by dst (quad-major implied)
        o1 = np.argsort(d_c, kind="stable")
        # layer-2/3 stream: edges sorted by (quad, chunk, dst); self edges
        # are a synthetic chunk k=CH gathered from the local hw_shard.
        m23 = owner[:E] == c
        s23 = src[:E][m23]
        d23 = dst[:E][m23] - c * NC
        n23 = norm[:E][m23]
        ck23 = s23 // CHN
        vloc = np.arange(NC, dtype=np.int64)
        s23 = np.concatenate([s23, vloc])
        d23 = np.concatenate([d23, vloc])
        n23 = np.concatenate([n23, dinv2[c * NC + vloc]])
        ck23 = np.concatenate([ck23, np.full(NC, CH, np.int64)])
        q23 = d23 // 512
        o23 = np.lexsort((d23, ck23, q23))
        srel23 = np.where(ck23 == CH, s23, s23 - ck23 * CHN)
        per_core.append(dict(
            s=src[m], d=d_c, n=norm[m], xs=xsrc[m], o1=o1,
            s23=srel23, d23=d23, n23=n23, ck23=ck23, o23=o23))

    # ---- uniform tile counts (maxed across cores) ----
    t1_q = np.zeros(Q, dtype=np.int64)
    t23_qk = np.zeros((Q, CHX), dtype=np.int64)
    for c in range(C):
        pc = per_core[c]
        q1 = pc["d"][pc["o1"]] // 512
        cnt1 = np.bincount(q1, minlength=Q)
        t1_q = np.maximum(t1_q, -(-cnt1 // P))
        dk = pc["d23"][pc["o23"]]
        kk = pc["ck23"][pc["o23"]]
        gid = (dk // 512) * CHX + kk
        cntk = np.bincount(gid, minlength=Q * CHX).reshape(Q, CHX)
        t23_qk = np.maximum(t23_qk, -(-cntk // P))
    assert (t1_q >= 1).all()
    assert (t23_qk >= 1).all()

    st = Structure(t1_q=list(t1_q), t23_qk=[list(r) for r in t23_qk])

    # ---- call lists ----
    # layer 1: per-quad TPC-tile chops, stream order = (q, tile)
    calls1 = []
    toff = 0
    for q in range(Q):
        tq = int(t1_q[q])
        t = 0
        while t < tq:
            nt = min(cfg.TPC, tq - t)
            calls1.append((q, toff + t, nt))
            t += nt
        toff += tq
    st.calls1 = calls1
    # layers 2/3: one call per (q, k)
    calls23 = []
    toff = 0
    for q in range(Q):
        for k in range(CHX):
            tqk = int(t23_qk[q][k])
            calls23.append((q, k, toff, tqk))
            toff += tqk
    st.calls23 = calls23

    # ---- build padded per-core streams ----
    def build_stream(d_sorted, n_sorted, idx_sorted, group_of_edge,
                     counts_T, n_groups):
        """Pack a sorted edge stream into per-group padded tiles.

        group_of_edge: group ordinal per edge (sorted ascending)
        counts_T: tiles per group ordinal
        returns meta [P, Ttot, 2] f32 (dstrel%128, norm), idxs [Ttot*128] i16
        """
        Ttot = int(sum(counts_T))
        meta = np.zeros((P, Ttot, 2), dtype=np.float32)
        idxs = np.zeros(Ttot * P, dtype=np.int16)
        cnt = np.bincount(group_of_edge, minlength=n_groups)
        starts = np.concatenate([[0], np.cumsum(cnt)[:-1]])
        t0 = 0
        for g in range(n_groups):
            cg, sg, Tg = int(cnt[g]), int(starts[g]), int(counts_T[g])
            assert cg <= Tg * P, (g, cg, Tg)
            sl = slice(sg, sg + cg)
            ii = np.arange(cg)
            tt = t0 + ii // P
            pp = ii % P
            meta[pp, tt, 0] = (d_sorted[sl] % 512).astype(np.float32)
            meta[pp, tt, 1] = n_sorted[sl]
            idxs[t0 * P + ii] = idx_sorted[sl].astype(np.int16)
            t0 += Tg
        return meta, idxs

    def wrap_idxs(idx_stream, calls, nidx_cols):
        out = np.zeros((P, nidx_cols), dtype=np.int16)
        col = 0
        for (t0, nt) in calls:
            arr = idx_stream[t0 * P:(t0 + nt) * P]
            wr = arr.reshape(-1, 16).T  # [16, nt*8]
            out[:, col:col + nt * 8] = np.tile(wr, (8, 1))
            col += nt * 8
        assert col == nidx_cols
        return out

    T1, T23 = st.T1, st.T23

    # group ordinal maps (stream order)
    counts1 = [int(t) for t in t1_q]
    counts23 = []
    for q in range(Q):
        for k in range(CHX):
            counts23.append(int(t23_qk[q][k]))

    in_maps = []
    for c in range(C):
        pc = per_core[c]
        o1 = pc["o1"]
        d1, n1, xs1 = pc["d"][o1], pc["n"][o1], pc["xs"][o1]
        g1 = d1 // 512
        meta1, idx1 = build_stream(d1, n1, xs1, g1, counts1, Q)

        o23 = pc["o23"]
        dk, nk, srel, kk = (pc["d23"][o23], pc["n23"][o23],
                            pc["s23"][o23], pc["ck23"][o23])
        g23 = (dk // 512) * CHX + kk
        assert (np.diff(g23) >= 0).all()
        meta23, idx23 = build_stream(dk, nk, srel, g23, counts23, Q * CHX)

        gidx1 = wrap_idxs(idx1, [(t0, nt) for (_q, t0, nt) in st.calls1],
                          T1 * 8)
        gidx23 = wrap_idxs(idx23, [(t0, nt) for (_q, _k, t0, nt)
                                   in st.calls23], T23 * 8)

        # pooling metadata
        nodes = np.arange(cfg.NCP) + c * NC
        valid = nodes < (c + 1) * NC
        bvals = np.where(valid, batch[np.minimum(nodes, N - 1)], -1)
        gmin = int(batch[c * NC])
        gmax = int(batch[min((c + 1) * NC, N) - 1])
        assert gmax - gmin < cfg.GSPAN, (c, gmin, gmax)
        brel = np.where(valid, bvals - gmin, -1).astype(np.float32)
        pool_meta = brel.reshape(cfg.W, P).T.copy()  # [128, W]
        gid_rows = gmin + np.arange(cfg.GSPAN)
        gid_rows = np.where(gid_rows < cfg.G, gid_rows,
                            cfg.G + np.arange(cfg.GSPAN) % 256).astype(np.int32)
        gid_cols = gid_rows.reshape(2, P).T.copy()  # [128, 2]

        cnts = np.bincount(batch, minlength=cfg.G).astype(np.float32)
        recip = 1.0 / np.maximum(cnts, 1.0)
        recip_pm = recip.reshape(cfg.G // P, P).T.copy()

        in_maps.append({
            "meta1": meta1, "gidx1": gidx1,
            "meta23": meta23, "gidx23": gidx23,
            "pool_meta": pool_meta, "gid_cols": gid_cols,
            "recip_pm": recip_pm,
            "emb": np.asarray(emb_table, dtype=np.float32),
            "Ws": np.asarray(Ws, dtype=np.float32),
            "bs": np.asarray(bs, dtype=np.float32),
        })

    return st, in_maps


# --------------------------------------------------------------------------
# device program
# --------------------------------------------------------------------------

DEBUG_STAGE = 0  # 0=off; 1=dump hT after layer1; 2=after layer2; 3=after layer3


def build_nc(cfg: Cfg, st: Structure):
    N, H, C, CH, W, Q = cfg.N, cfg.H, cfg.C, cfg.CH, cfg.W, cfg.Q
    NC, CHN, NCP = cfg.NC, cfg.CHN, cfg.NCP
    CHX = CH + 1
    T1, T23 = st.T1, st.T23
    GS = cfg.GSPAN
    GW = cfg.G // P
    NQ = cfg.NQ
    TQK = max(nt for (_q, _k, _t0, nt) in st.calls23)

    nc = bacc.Bacc(None, num_devices=C, num_swdge_queues=NQ)
    cores = list(range(C))

    # ---- external I/O ----
    meta1 = nc.declare_dram_parameter("meta1", [P, T1, 2], F32, isOutput=False)
    gidx1 = nc.declare_dram_parameter("gidx1", [P, T1 * 8], I16, isOutput=False)
    meta23 = nc.declare_dram_parameter("meta23", [P, T23, 2], F32, isOutput=False)
    gidx23 = nc.declare_dram_parameter("gidx23", [P, T23 * 8], I16, isOutput=False)
    pool_meta = nc.declare_dram_parameter("pool_meta", [P, W], F32, isOutput=False)
    gid_cols = nc.declare_dram_parameter("gid_cols", [P, 2], I32, isOutput=False)
    recip_pm = nc.declare_dram_parameter("recip_pm", [P, GW], F32, isOutput=False)
    emb_d = nc.declare_dram_parameter("emb", [P, H], F32, isOutput=False)
    Ws_d = nc.declare_dram_parameter("Ws", [cfg.L, H, H], F32, isOutput=False)
    bs_d = nc.declare_dram_parameter("bs", [cfg.L, H], F32, isOutput=False)
    out_d = nc.declare_dram_parameter("out", [cfg.G, H], F32, isOutput=True)

    # ---- internal DRAM ----
    t1_dram = nc.dram_tensor("t1_tab", [cfg.V, H], BF16)
    hw_shard = nc.dram_tensor("hw_shard", [NC, H], BF16)
    hw_full = nc.dram_tensor("hw_full", [N, H], BF16, addr_space="Shared")
    pooled_nm = nc.dram_tensor("pooled_nm", [cfg.G + GS, H], F32)
    pooled_sum = nc.dram_tensor("pooled_sum", [cfg.G + GS, H], F32,
                                addr_space="Shared")

    from concourse.tile import add_dep_helper
    pd = {"i": 0, "last": None}

    def chain_pool_dma(inst):
        if pd["last"] is not None:
            add_dep_helper(inst.ins, pd["last"].ins, sync=False,
                           reason="pool-dma queue/lane parity order")
        pd["last"] = inst
        pd["i"] += 1

    with tile.TileContext(nc) as tc, ExitStack() as ctx:
        const = ctx.enter_context(tc.tile_pool(name="const", bufs=1))
        hpool = ctx.enter_context(tc.tile_pool(name="hbuf", bufs=1))

        ident = const.tile([P, P], F32)
        make_identity(nc, ident[:])
        ident_bf = const.tile([P, P], BF16)
        make_identity(nc, ident_bf[:])
        iota_i = const.tile([P, 512], I32)
        nc.gpsimd.iota(iota_i[:], pattern=[[1, 512]], base=0,
                       channel_multiplier=0)
        iota_bf = const.tile([P, GS], BF16)
        nc.vector.tensor_copy(out=iota_bf[:], in_=iota_i[:, :GS])
        iota_f512 = const.tile([P, 512], F32)
        nc.vector.tensor_copy(out=iota_f512[:], in_=iota_i[:])

        w_sb = const.tile([P, H], F32, tag="w_sb")
        w_bf = const.tile([P, H], BF16, tag="w_bf")
        emb_sb = const.tile([P, H], F32, tag="w_sb2")
        b_cols = const.tile([P, cfg.L], F32)
        for l in range(cfg.L):
            nc.sync.dma_start(out=b_cols[:, l:l + 1], in_=bs_d[l, :, None])

        hT_a = hpool.tile([P, NCP], BF16)   # layer outputs (ping)
        hT_b = hpool.tile([P, NCP], BF16)   # (pong)

        # ---------------- T1 = emb @ W1 (bf16 table) ----------------
        with tc.tile_pool(name="pro", bufs=2) as pro, \
             tc.tile_pool(name="pro_ps", bufs=2, space="PSUM") as pro_ps:
            nc.sync.dma_start(out=emb_sb[:], in_=emb_d[:, :])
            nc.sync.dma_start(out=w_sb[:], in_=Ws_d[0])
            embT_ps = pro_ps.tile([P, P], F32)
            nc.tensor.transpose(out=embT_ps[:], in_=emb_sb[:], identity=ident[:])
            embT = pro.tile([P, P], F32)
            nc.vector.tensor_copy(out=embT[:], in_=embT_ps[:])
            t1t_ps = pro_ps.tile([P, P], F32)
            nc.tensor.matmul(out=t1t_ps[:], lhsT=w_sb[:], rhs=embT[:],
                             start=True, stop=True)
            t1t = pro.tile([P, P], F32)
            nc.vector.tensor_copy(out=t1t[:], in_=t1t_ps[:])
            t1nm_ps = pro_ps.tile([P, P], F32)
            nc.tensor.transpose(out=t1nm_ps[:], in_=t1t[:], identity=ident[:])
            t1nm = pro.tile([P, P], BF16)
            nc.vector.tensor_copy(out=t1nm[:], in_=t1nm_ps[:])
            nc.sync.dma_start(out=t1_dram[:, :], in_=t1nm[:])

        # ---------------- edge pass ----------------
        def edge_pass(layer, h_out):
            l1 = layer == 0
            meta_d, gidx_d = (meta1, gidx1) if l1 else (meta23, gidx23)
            calls = ([(q, None, t0, nt) for (q, t0, nt) in st.calls1] if l1
                     else st.calls23)
            Ttot = T1 if l1 else T23
            MAXT = max(nt for (_q, _k, _t0, nt) in calls)

            with tc.tile_pool(name=f"ix{layer}", bufs=1) as ixp, \
                 tc.tile_pool(name=f"ep{layer}", bufs=8) as ep, \
                 tc.tile_pool(name=f"gb{layer}", bufs=8) as gb, \
                 tc.tile_pool(name=f"bq{layer}", bufs=4) as bq, \
                 tc.tile_pool(name=f"eps{layer}", bufs=2, space="PSUM") as eps:

                # preload the whole wrapped index array so gathers never
                # wait on per-call idx DMAs (lets the ucode batch queues)
                idxall = ixp.tile([P, Ttot * 8], I16, tag="idxall")
                IXC = 8192
                for s0 in range(0, Ttot * 8, IXC):
                    nn = min(IXC, Ttot * 8 - s0)
                    nc.sync.dma_start(out=idxall[:, s0:s0 + nn],
                                      in_=gidx_d[:, s0:s0 + nn])

                # issue all gathers + meta loads, keyed by call index
                gbuf, mbuf = {}, {}
                idxcol = 0
                for ci, (q, k, t0, nt) in enumerate(calls):
                    mb = ep.tile([P, MAXT, 2], F32, tag="meta")
                    nc.sync.dma_start(out=mb[:, :nt, :],
                                      in_=meta_d[:, t0:t0 + nt, :])
                    g = gb.tile([P, MAXT, H], BF16, tag="gath")
                    if l1:
                        src_ap = t1_dram[:, :]
                    elif k == CH:
                        src_ap = hw_shard[:, :]
                    else:
                        src_ap = hw_full[k * CHN:(k + 1) * CHN, :]
                    gi = nc.gpsimd.dma_gather(
                        out_ap=g[:, :nt, :], in_ap=src_ap,
                        idxs_ap=idxall[:, idxcol:idxcol + nt * 8],
                        num_idxs=nt * P, num_idxs_reg=nt * P,
                        elem_size=H, single_packet=False,
                        queue_num=pd["i"] % NQ)
                    chain_pool_dma(gi)
                    gbuf[ci] = g
                    mbuf[ci] = mb
                    idxcol += nt * 8

                def emit_tile(ci, slot, qpsum, first, last):
                    g, mb = gbuf[ci], mbuf[ci]
                    B = bq.tile([P, 512], BF16, tag="B")
                    nc.vector.tensor_scalar(
                        out=B[:], in0=iota_f512[:],
                        scalar1=mb[:, slot, 0:1],
                        scalar2=mb[:, slot, 1:2],
                        op0=OP.is_equal, op1=OP.mult)
                    nc.tensor.matmul(
                        out=qpsum[:], lhsT=g[:, slot, :], rhs=B[:],
                        start=first, stop=last)

                ci_of = {}
                for ci, (q, k, t0, nt) in enumerate(calls):
                    ci_of.setdefault((q, k), []).append(ci)

                func = AF.Relu if layer < cfg.L - 1 else AF.Identity
                for q in range(Q):
                    nw = min(4, W - q * 4)
                    qpsum = eps.tile([P, 512], F32, tag="qp")
                    if l1:
                        tq = st.t1_q[q]
                        cis = ci_of[(q, None)]
                        sizes = [calls[ci][3] for ci in cis]
                        pos = 0
                        for i in range(tq):
                            rem = pos
                            for ci, sz in zip(cis, sizes):
                                if rem < sz:
                                    break
                                rem -= sz
                            emit_tile(ci, rem, qpsum, i == 0, i == tq - 1)
                            pos += 1
                    else:
                        tq = sum(st.t23_qk[q])
                        done = 0
                        for k in range(CHX):
                            (ci,) = ci_of[(q, k)]
                            for i in range(st.t23_qk[q][k]):
                                emit_tile(ci, i, qpsum,
                                          done == 0, done == tq - 1)
                                done += 1
                    ncol = nw * P
                    nc.scalar.activation(
                        out=h_out[:, q * 512:q * 512 + ncol],
                        in_=qpsum[:, :ncol], func=func,
                        bias=b_cols[:, layer:layer + 1], scale=1.0)

        # ---------------- hw phase ----------------
        def hw_phase(layer, h_in):
            with tc.tile_pool(name=f"hw{layer}", bufs=3) as hp, \
                 tc.tile_pool(name=f"hwps{layer}", bufs=2, space="PSUM") as hps, \
                 tc.tile_pool(name=f"hwps2{layer}", bufs=2, space="PSUM") as hps2:
                nc.sync.dma_start(out=w_sb[:], in_=Ws_d[layer])
                nc.vector.tensor_copy(out=w_bf[:], in_=w_sb[:])
                CWW = 512
                for j0 in range(0, NC, CWW):
                    nj = min(CWW, NC - j0)
                    ps = hps.tile([P, CWW], F32, tag="mm")
                    nc.tensor.matmul(out=ps[:, :nj], lhsT=w_bf[:],
                                     rhs=h_in[:, j0:j0 + nj],
                                     start=True, stop=True)
                    hw_s = hp.tile([P, CWW], F32, tag="hw_s")
                    nc.scalar.activation(out=hw_s[:, :nj], in_=ps[:, :nj],
                                         func=AF.Copy)
                    for q0 in range(0, nj, P):
                        nq = min(P, nj - q0)
                        pt = hps2.tile([P, P], F32, tag="tr")
                        nc.tensor.transpose(out=pt[:nq, :],
                                            in_=hw_s[:, q0:q0 + nq],
                                            identity=ident[:])
                        stg = hp.tile([P, P], BF16, tag="stg")
                        nc.scalar.activation(out=stg[:nq, :], in_=pt[:nq, :],
                                             func=AF.Copy)
                        nc.sync.dma_start(
                            out=hw_shard[j0 + q0:j0 + q0 + nq, :],
                            in_=stg[:nq, :])
            nc.gpsimd.collective_compute(
                "AllGather", OP.bypass, replica_groups=[cores],
                ins=[hw_shard[:, :]], outs=[hw_full[:, :]])

        def dump_h(src_tile):
            dbg_d = nc.declare_dram_parameter("dbg", [P, NCP], F32,
                                              isOutput=True)
            with tc.tile_pool(name="dbg", bufs=2) as dbp:
                CWD = 512
                for s0 in range(0, NCP, CWD):
                    nn = min(CWD, NCP - s0)
                    dt_ = dbp.tile([P, CWD], F32, tag="d")
                    nc.vector.tensor_copy(out=dt_[:, :nn],
                                          in_=src_tile[:, s0:s0 + nn])
                    nc.sync.dma_start(out=dbg_d[:, s0:s0 + nn],
                                      in_=dt_[:, :nn])

        with nc.named_scope("layer1"):
            edge_pass(0, hT_a)
        if DEBUG_STAGE == 1:
            dump_h(hT_a)
        with nc.named_scope("hw2"):
            hw_phase(1, hT_a)
        with nc.named_scope("layer2"):
            edge_pass(1, hT_b)
        if DEBUG_STAGE == 2:
            dump_h(hT_b)
        with nc.named_scope("hw3"):
            hw_phase(2, hT_b)
        with nc.named_scope("layer3"):
            edge_pass(2, hT_a)
        if DEBUG_STAGE == 3:
            dump_h(hT_a)

        # ---------------- pooling ----------------
        with nc.named_scope("pool"), \
             tc.tile_pool(name="po", bufs=3) as po, \
             tc.tile_pool(name="po_ps", bufs=2, space="PSUM") as po_ps, \
             tc.tile_pool(name="po_acc", bufs=1, space="PSUM") as po_acc:
            pm = po.tile([P, W], F32, tag="pm")
            nc.sync.dma_start(out=pm[:], in_=pool_meta[:, :])
            gcols = po.tile([P, 2], I32, tag="gcols")
            nc.sync.dma_start(out=gcols[:], in_=gid_cols[:, :])
            recip_sb = po.tile([P, GW], F32, tag="recip")
            nc.sync.dma_start(out=recip_sb[:], in_=recip_pm[:, :])

            acc = po_acc.tile([P, GS], F32)
            for t in range(W):
                pt = po_ps.tile([P, P], BF16, tag="ptr")
                nc.tensor.transpose(out=pt[:], in_=hT_a[:, t * P:(t + 1) * P],
                                    identity=ident_bf[:])
                h3nm = po.tile([P, P], BF16, tag="h3nm")
                nc.scalar.activation(out=h3nm[:], in_=pt[:], func=AF.Copy)
                Bp = po.tile([P, GS], BF16, tag="Bp")
                nc.vector.tensor_scalar(
                    out=Bp[:], in0=iota_bf[:],
                    scalar1=pm[:, t:t + 1], scalar2=None,
                    op0=OP.is_equal)
                nc.tensor.matmul(out=acc[:], lhsT=h3nm[:], rhs=Bp[:],
                                 start=(t == 0), stop=(t == W - 1))

            def dummy_gather():
                dz = po.tile([P, 1, P], BF16, tag="dz")
                zi = po.tile([P, 8], I16, tag="zi")
                nc.vector.memset(zi[:], 0)
                gi = nc.gpsimd.dma_gather(
                    out_ap=dz[:], in_ap=t1_dram[:, :], idxs_ap=zi[:],
                    num_idxs=P, num_idxs_reg=P, elem_size=H,
                    single_packet=False, queue_num=pd["i"] % NQ)
                chain_pool_dma(gi)

            # zero pooled_nm
            zt = po.tile([P, P], F32, tag="zt")
            nc.vector.memset(zt[:], 0.0)
            for r0 in range(0, cfg.G + GS, P):
                nc.sync.dma_start(out=pooled_nm[r0:r0 + P, :], in_=zt[:])

            acc_sb = po.tile([P, GS], F32, tag="acc_sb")
            nc.scalar.activation(out=acc_sb[:], in_=acc[:], func=AF.Copy)
            for half in range(2):
                pt = po_ps.tile([P, P], F32, tag="ptr")
                nc.tensor.transpose(out=pt[:],
                                    in_=acc_sb[:, half * P:(half + 1) * P],
                                    identity=ident[:])
                rows = po.tile([P, P], F32, tag="rows")
                nc.scalar.activation(out=rows[:], in_=pt[:], func=AF.Copy)
                while pd["i"] % NQ != 0:
                    dummy_gather()  # scatters run on queue 0: align lane
                si = nc.gpsimd.indirect_dma_start(
                    out=pooled_nm[:, :],
                    out_offset=IndirectOffsetOnAxis(
                        ap=gcols[:, half:half + 1], axis=0),
                    in_=rows[:], in_offset=None)
                chain_pool_dma(si)

            nc.gpsimd.collective_compute(
                "AllReduce", OP.add, replica_groups=[cores],
                ins=[pooled_nm[:, :]], outs=[pooled_sum[:, :]])

            for gw in range(GW):
                ot = po.tile([P, H], F32, tag="ot")
                nc.sync.dma_start(out=ot[:],
                                  in_=pooled_sum[gw * P:(gw + 1) * P, :])
                os = po.tile([P, H], F32, tag="os")
                nc.vector.tensor_scalar(
                    out=os[:], in0=ot[:], scalar1=recip_sb[:, gw:gw + 1],
                    scalar2=None, op0=OP.mult)
                nc.sync.dma_start(out=out_d[gw * P:(gw + 1) * P, :],
                                  in_=os[:])

    return nc


# --------------------------------------------------------------------------
# entry point: full inputs -> full output
# --------------------------------------------------------------------------

_CACHE = {}


def _get_compiled(cfg, st_key, st):
    if st_key not in _CACHE:
        nc = build_nc(cfg, st)
        nc.finalize()
        _CACHE[st_key] = nc
    return _CACHE[st_key]


def kernel(x, edge_index, batch, emb_table, Ws, bs):
    cfg = Cfg()  # full problem size, hardcoded
    st, in_maps = preprocess(x, edge_index, batch, emb_table, Ws, bs, cfg)
    st_key = (tuple(st.t1_w), tuple(tuple(r) for r in st.t23_kw))
    nc = _get_compiled(cfg, st_key, st)

    from concourse.bass_utils import run_bass_kernel_spmd

    res = run_bass_kernel_spmd(nc, in_maps, list(range(cfg.C)))
    return np.ascontiguousarray(res.results[0]["out"])
